# revision 24
# baseline (speedup 1.0000x reference)
import sys

sys.path.insert(0, "/opt/trn_rl_repo")

import numpy as np

# Problem constants (hardcoded per contract)
B, L, C, K = 8, 16384, 64, 7
T = (L - 2 * K) // 2 + 1  # 8186
HALF = 4096               # t's per half (half-1 ragged: 8186-4096=4090, padded)
TC = 512                  # t-chunk
NCH = HALF // TC          # 8 chunks
WX = 4104                 # column width of folded x tensors (HALF + 8 pad)
LN_EPS = 1e-6

USE_INT8 = True
QMULT = 50.0              # int8 grid density (u = i/QMULT before expansion)
QA = 0.1                  # cubic companding: x = u*(1 + QA*u^2)

# f16 const block layout (f16 cols): WT | ID | CK
C_WT = 0
C_ID = C_WT + 64 * K       # 448
C_CK = C_ID + 128          # 576
NCONS = C_CK + 64          # 640

# int8 packed layout (bytes): XEI | XOI  (consts ride in the NEFF)
O8_XE = 0
O8_XO = WX                 # 4104
NB8 = 2 * WX               # 8208

# int8 output layout (bytes): quantized data | per-t f16 scales (bitcast)
# data col = i*512 + h*256 + j*64 + c ; scale col = i*8 + h*4 + j
OQ_SCL = NCH * 512         # 4096
NOB = OQ_SCL + 2 * 64      # 4224

# f16 packed layout (f16 cols): XE | XO | consts
O_XE = 0
O_XO = WX
O_CONS = 2 * WX
WIN = O_CONS + NCONS       # 8848

_CACHE = {}


def _install_compile_memo():
    """Memoize the BIR->NEFF compile inside the HLO compile hook. Repeated
    spmd calls re-lower the same module through fresh jit closures; the HLO
    bytes differ only in module id / source-line metadata, so without this
    every call re-runs the walrus subprocess + NEFF rename for an identical
    result. Key: sha256 of the HLO normalized (id=0, instruction metadata
    cleared). On a hit the cached NEFF bytes are re-wrapped around the
    *current* call's HLO, so the returned module stays per-call consistent."""
    import hashlib
    from concourse import bass2jax

    if getattr(bass2jax.neuronx_cc_hook, "_is_memo", False):
        return
    real = bass2jax.neuronx_cc_hook
    neff_memo = {}

    def hook(code, code_format, platform_version, file_prefix):
        try:
            import libneuronxla.proto.hlo_pb2 as hlo_pb2
            from libneuronxla.libncc import _wrap_neff_as_custom_call

            code_b = bytes(code)
            if b"bass_exec" not in code_b or bytes(code_format) != b"hlo":
                return real(code, code_format, platform_version, file_prefix)
            proto = hlo_pb2.HloModuleProto.FromString(code_b)
            is_exec = any(
                ins.opcode == "custom-call" and ins.custom_call_target == "bass_exec"
                for comp in proto.computations for ins in comp.instructions)
            if not is_exec:
                return real(code, code_format, platform_version, file_prefix)
            norm = hlo_pb2.HloModuleProto()
            norm.CopyFrom(proto)
            norm.id = 0
            for comp in norm.computations:
                for ins in comp.instructions:
                    ins.ClearField("metadata")
            key = (hashlib.sha256(norm.SerializeToString(deterministic=True))
                   .digest(), str(platform_version))
            neff = neff_memo.get(key)
            if neff is not None:
                return 0, _wrap_neff_as_custom_call(code_b, neff)
            r = real(code, code_format, platform_version, file_prefix)
            try:
                wrapped = hlo_pb2.HloModuleProto.FromString(bytes(r[1]))
                for comp in wrapped.computations:
                    for ins in comp.instructions:
                        if (ins.opcode == "custom-call"
                                and ins.custom_call_target == "AwsNeuronNeff"):
                            neff_memo[key] = ins.backend_config
            except Exception:
                pass
            return r
        except Exception:
            return real(code, code_format, platform_version, file_prefix)

    hook._is_memo = True
    bass2jax.neuronx_cc_hook = hook
    try:
        import libneuronxla
        if getattr(libneuronxla, "neuronx_cc", None) is real:
            libneuronxla.neuronx_cc = hook
    except ImportError:
        pass


def _build(prelu_slope: float, need_lnsb: bool, need_cb: bool, use_int8: bool,
           cons_data=None, sq: float = 1.0):
    import concourse.bacc as bacc
    import concourse.mybir as mybir
    import concourse.tile as tile

    f32 = mybir.dt.float32
    f16 = mybir.dt.float16
    i8 = mybir.dt.int8
    AF = mybir.ActivationFunctionType
    OP = mybir.AluOpType

    nc = bacc.Bacc("TRN2", target_bir_lowering=False, debug=False, num_devices=8)

    # ---- DRAM parameters (per-core shard data) ----
    if use_int8:
        dIN = nc.declare_dram_parameter("inp", [128, NB8], i8, isOutput=False)
    else:
        dIN = nc.declare_dram_parameter("inp", [128, WIN], f16, isOutput=False)
    if need_lnsb or need_cb:
        dCST = nc.declare_dram_parameter("csts", [128, 4], f32, isOutput=False)
    if use_int8:
        dOUT = nc.declare_dram_parameter("out", [128, NOB], i8, isOutput=True)
    else:
        dOUT = nc.declare_dram_parameter("out", [128, HALF], f16, isOutput=True)

    from contextlib import ExitStack

    with ExitStack() as es:
        tc = es.enter_context(tile.TileContext(nc))
        cp = es.enter_context(tc.tile_pool(name="const", bufs=1))
        gp = es.enter_context(tc.tile_pool(name="gps", bufs=2, space="PSUM"))
        yp = es.enter_context(tc.tile_pool(name="yps", bufs=1, space="PSUM"))
        zp = es.enter_context(tc.tile_pool(name="zps", bufs=1, space="PSUM"))
        sp = es.enter_context(tc.tile_pool(name="sps", bufs=1, space="PSUM"))
        hp = es.enter_context(tc.tile_pool(name="hsb", bufs=10))
        pp = es.enter_context(tc.tile_pool(name="prod", bufs=16))
        ypool = es.enter_context(tc.tile_pool(name="ysb", bufs=3))
        st1 = es.enter_context(tc.tile_pool(name="st1", bufs=3))
        st2 = es.enter_context(tc.tile_pool(name="st2", bufs=3))
        st3 = es.enter_context(tc.tile_pool(name="st3", bufs=3))
        st4 = es.enter_context(tc.tile_pool(name="st4", bufs=3))
        st5 = es.enter_context(tc.tile_pool(name="st5", bufs=3))
        ynp = es.enter_context(tc.tile_pool(name="ynp", bufs=3))
        pzp = es.enter_context(tc.tile_pool(name="pzp", bufs=3))
        op_ = es.enter_context(tc.tile_pool(name="outp", bufs=4))
        trp = es.enter_context(tc.tile_pool(name="trp", bufs=4))
        qp = es.enter_context(tc.tile_pool(name="qp", bufs=3))
        smx = es.enter_context(tc.tile_pool(name="smx", bufs=24))
        if True:
            # ---- load packed input; dequant / slice constituents ----
            if use_int8:
                dCONS = nc.inline_tensor(cons_data, "cons")
                XEI = cp.tile([128, WX], i8)
                XOI = cp.tile([128, WX], i8)
                CONS = cp.tile([128, NCONS], f16)
                nc.sync.dma_start(XEI[:], dIN[:, O8_XE:O8_XE + WX])
                nc.sync.dma_start(XOI[:], dIN[:, O8_XO:O8_XO + WX])
                nc.sync.dma_start(CONS[:], dCONS[:])
                # companded decode: u = i*s ; x = u*(1 + alpha*u^2)
                XE = cp.tile([128, WX], f16)
                XO = cp.tile([128, WX], f16)
                scl = float(sq / QMULT)
                alpha = float(QA / (sq * sq))
                for XU, XI in ((XE, XEI), (XO, XOI)):
                    u = cp.tile([128, WX], f16)
                    nc.scalar.activation(u[:], XI[:], AF.Copy, scale=scl)
                    u2 = cp.tile([128, WX], f16)
                    nc.vector.tensor_mul(u2[:], u[:], u[:])
                    g2 = cp.tile([128, WX], f16)
                    nc.vector.tensor_scalar(g2[:], u2[:], alpha, 1.0,
                                            op0=OP.mult, op1=OP.add)
                    nc.vector.tensor_mul(XU[:], u[:], g2[:])
                XE, XO = XE[:], XO[:]
            else:
                IN = cp.tile([128, WIN], f16)
                nc.sync.dma_start(IN[:], dIN[:])
                XE = IN[:, O_XE:O_XE + WX]
                XO = IN[:, O_XO:O_XO + WX]
                CONS = IN[:, O_CONS:O_CONS + NCONS]
            WT = CONS[:, C_WT:C_WT + 64 * K]
            ID = CONS[:, C_ID:C_ID + 128]
            CKt = CONS[:, C_CK:C_CK + 64]
            ON = cp.tile([128, 64], f16)
            nc.vector.memset(ON[:], 1.0 / 64)
            EPS = cp.tile([128, 1], f32)
            nc.vector.memset(EPS[:], LN_EPS)
            if use_int8:
                SCL = cp.tile([128, 64], f16)
            if need_lnsb or need_cb:
                CST = cp.tile([128, 4], f32)
                nc.sync.dma_start(CST[:], dCST[:])

            for i in range(NCH):
                t0 = TC * i
                # ---- G matmuls + tanh: 7 m-planes, each (Ge|Go) (128,1024) ----
                hts = []
                for m in range(K):
                    g = gp.tile([128, 1024], f32)
                    for ci, src_ in ((0, XE), (512, XO)):
                        for h in (0, 1):
                            p0 = 64 * h
                            nc.tensor.matmul(
                                g[p0:p0 + 64, ci:ci + TC],
                                lhsT=WT[p0:p0 + 64, 64 * m:64 * m + 64],
                                rhs=src_[p0:p0 + 64, t0 + 6:t0 + 6 + TC],
                                start=True, stop=True,
                            )
                    ht = hp.tile([128, 1024], f16)
                    nc.scalar.activation(ht[:], g[:], AF.Tanh)
                    hts.append(ht)

                # ---- gating products (14 planes) ----
                prods = []
                for m in range(K):
                    for ci, xa in ((0, XE), (512, XO)):
                        pr = pp.tile([128, TC], f16)
                        nc.vector.tensor_mul(pr[:], xa[:, t0 + m:t0 + m + TC],
                                             hts[m][:, ci:ci + TC])
                        prods.append(pr)

                # ---- accumulate 14 products + skip via identity matmuls ----
                y = yp.tile([128, TC], f32)
                for j, pr in enumerate(prods):
                    nc.tensor.matmul(y[:], lhsT=ID, rhs=pr[:],
                                     start=(j == 0), stop=False)
                nc.tensor.matmul(y[:], lhsT=ID,
                                 rhs=XE[:, t0 + 6:t0 + 6 + TC],
                                 start=False, stop=True)

                # ---- drain y, square ----
                ysb = ypool.tile([128, TC], f16)
                nc.scalar.copy(ysb[:], y[:])
                ysq = pp.tile([128, TC], f16)
                nc.vector.tensor_mul(ysq[:], ysb[:], ysb[:])

                # ---- LN stats: mean & mean-of-squares via ones-matmul ----
                st = sp.tile([128, 1024], f32)
                for h in (0, 1):
                    p0 = 64 * h
                    nc.tensor.matmul(st[p0:p0 + 64, 0:TC],
                                     lhsT=ON[p0:p0 + 64, :],
                                     rhs=ysb[p0:p0 + 64, :], start=True, stop=True)
                    nc.tensor.matmul(st[p0:p0 + 64, 512:512 + TC],
                                     lhsT=ON[p0:p0 + 64, :],
                                     rhs=ysq[p0:p0 + 64, :], start=True, stop=True)
                mu = st[:, 0:TC]
                m2 = st[:, 512:512 + TC]

                musq = st1.tile([128, TC], f32)
                nc.scalar.activation(musq[:], mu, AF.Square)
                var = st2.tile([128, TC], f32)
                nc.vector.tensor_sub(var[:], m2, musq[:])
                std = st3.tile([128, TC], f32)
                nc.scalar.activation(std[:], var[:], AF.Sqrt, bias=EPS[:, 0:1])
                rstd = st4.tile([128, TC], f32)
                scr = st5.tile([128, TC], f32)
                nc.vector.reciprocal_approx_accurate(rstd[:], std[:], scr[:])

                # ---- yn = (y - mu) * rstd  (* s + b) ----
                yc = st1.tile([128, TC], f32)
                nc.vector.tensor_sub(yc[:], ysb[:], mu)
                yn = ynp.tile([128, TC], f16)
                nc.vector.tensor_mul(yn[:], yc[:], rstd[:])
                if need_lnsb:
                    yn2 = ynp.tile([128, TC], f16)
                    nc.vector.tensor_scalar(yn2[:], yn[:], CST[:, 0:1], CST[:, 1:2],
                                            op0=OP.mult, op1=OP.add)
                    yn = yn2

                # ---- 1x1 conv ----
                z = zp.tile([128, TC], f32)
                for h in (0, 1):
                    p0 = 64 * h
                    nc.tensor.matmul(z[p0:p0 + 64, :], lhsT=CKt[p0:p0 + 64, :],
                                     rhs=yn[p0:p0 + 64, :], start=True, stop=True)
                if need_cb:
                    z2 = st2.tile([128, TC], f32)
                    nc.vector.tensor_scalar(z2[:], z[:], CST[:, 2:3], None, op0=OP.add)
                    zsrc = z2
                else:
                    zsrc = z
                # prelu: max(z, slope*z)
                pz = pzp.tile([128, TC], f16)
                nc.scalar.activation(pz[:], zsrc[:], AF.Prelu,
                                     alpha=float(prelu_slope))

                # ---- out = yn + pz ----
                of = op_.tile([128, TC], f16)
                nc.vector.tensor_add(of[:], yn[:], pz[:])
                if not use_int8:
                    nc.sync.dma_start(dOUT[:, t0:t0 + TC], of[:])
                    continue

                # ---- int8 store: transpose to t-major, per-t absmax quant ----
                AX = mybir.AxisListType
                qb = qp.tile([128, 2, 4, 64], i8)
                for h in (0, 1):
                    p0 = 64 * h
                    ofT = trp.tile([128, 4, 64], f16)
                    nc.sync.dma_start_transpose(ofT[:], of[p0:p0 + 64, :])
                    mx = smx.tile([128, 4], f32)
                    nc.vector.tensor_reduce(mx[:], ofT[:], axis=AX.X, op=OP.max,
                                            apply_absolute_value=True)
                    mxe = smx.tile([128, 4], f32)
                    nc.vector.tensor_scalar(mxe[:], mx[:], EPS[:, 0:1], None,
                                            op0=OP.add)
                    # dequant scale s = (absmax+eps)/127 stored as f16; the
                    # quant multiplier is 1/float32(s) so host dequant with s
                    # is the exact inverse (up to the recip approximation).
                    scol = 8 * i + 4 * h
                    nc.vector.tensor_scalar(SCL[:, scol:scol + 4], mxe[:],
                                            1.0 / 127.0, None, op0=OP.mult)
                    sf = smx.tile([128, 4], f32)
                    nc.scalar.copy(sf[:], SCL[:, scol:scol + 4])
                    inv = smx.tile([128, 4], f32)
                    scr = smx.tile([128, 4], f32)
                    nc.vector.reciprocal_approx_accurate(inv[:], sf[:], scr[:])
                    for j in range(4):
                        nc.vector.tensor_scalar(qb[:, h, j, :], ofT[:, j, :],
                                                inv[:, j:j + 1], None,
                                                op0=OP.mult)
                dst = dOUT[:, 512 * i:512 * (i + 1)].rearrange(
                    "p (h j c) -> p h j c", h=2, j=4)
                nc.sync.dma_start(dst, qb[:])

            if use_int8:
                nc.sync.dma_start(dOUT[:, OQ_SCL:NOB].bitcast(f16), SCL[:])

    nc.compile()
    return nc


def _make_consts(weights, conv_kernel):
    CONS = np.zeros((128, NCONS), np.float16)
    for m in range(K):
        wmT = np.asarray(weights[:, :, m]).T.astype(np.float16)  # (c_in, d)
        CONS[0:64, C_WT + 64 * m:C_WT + 64 * m + 64] = wmT
        CONS[64:128, C_WT + 64 * m:C_WT + 64 * m + 64] = wmT
    CONS[:, C_ID:C_ID + 128] = np.eye(128, dtype=np.float16)
    ckc = np.asarray(conv_kernel).astype(np.float16)  # (c, o), lhsT layout
    CONS[0:64, C_CK:C_CK + 64] = ckc
    CONS[64:128, C_CK:C_CK + 64] = ckc
    return CONS


def _input_sigma(xf):
    """Power-of-two snap of std(x): keys the compiled decode constants."""
    sd = float(np.std(xf))
    if not np.isfinite(sd) or sd <= 0:
        return 1.0
    return float(2.0 ** round(np.log2(sd)))


def _encode_table(sq):
    """Decoded value per int8 code, mirroring the device's f16 op chain."""
    i = np.arange(-127, 128, dtype=np.float32)
    u = (i * np.float32(sq / QMULT)).astype(np.float16).astype(np.float32)
    u2 = (u * u).astype(np.float16).astype(np.float32)
    g = (np.float32(QA / (sq * sq)) * u2 + 1.0).astype(np.float16).astype(np.float32)
    return (u * g).astype(np.float16).astype(np.float32)


def _encode_int8(xb, table):
    mid = (table[1:] + table[:-1]) * 0.5
    idx = np.searchsorted(mid, xb.ravel()).astype(np.int16)
    return (idx - 127).astype(np.int8).reshape(xb.shape)


def _prep_inputs(x, weights, ln_scale, ln_bias, conv_kernel, conv_bias):
    """Host-side prep: returns per-core input maps."""
    xf = np.asarray(x, dtype=np.float32)
    CONS = _make_consts(weights, conv_kernel)
    if USE_INT8:
        table = _encode_table(_input_sigma(xf))

    need_cst = not (np.allclose(np.asarray(ln_scale), 1.0)
                    and np.allclose(np.asarray(ln_bias), 0.0)
                    and np.allclose(np.asarray(conv_bias), 0.0))
    CST = None
    if need_cst:
        CST = np.zeros((128, 4), np.float32)
        s = np.asarray(ln_scale, np.float32)
        b = np.asarray(ln_bias, np.float32)
        cb = np.asarray(conv_bias, np.float32)
        CST[0:64, 0] = s
        CST[64:128, 0] = s
        CST[0:64, 1] = b
        CST[64:128, 1] = b
        CST[0:64, 2] = cb
        CST[64:128, 2] = cb

    in_maps = []
    for bi in range(B):
        xb = xf[bi]                      # (L, C)
        if USE_INT8:
            xq = _encode_int8(xb, table)
            xeT = np.ascontiguousarray(xq[0::2].T)   # (64, 8192) int8
            xoT = np.ascontiguousarray(xq[1::2].T)
            IN = np.zeros((128, NB8), np.int8)
            for off, a in ((O8_XE, xeT), (O8_XO, xoT)):
                IN[0:64, off:off + WX] = a[:, 0:WX]
                IN[64:128, off:off + 8192 - HALF] = a[:, HALF:8192]
        else:
            xeT = np.ascontiguousarray(xb[0::2].T).astype(np.float16)
            xoT = np.ascontiguousarray(xb[1::2].T).astype(np.float16)
            IN = np.zeros((128, WIN), np.float16)
            for off, a in ((O_XE, xeT), (O_XO, xoT)):
                IN[0:64, off:off + WX] = a[:, 0:WX]
                IN[64:128, off:off + 8192 - HALF] = a[:, HALF:8192]
            IN[:, O_CONS:O_CONS + NCONS] = CONS
        m = {"inp": IN}
        if need_cst:
            m["csts"] = CST
        in_maps.append(m)
    return in_maps


def kernel(x, weights, ln_scale, ln_bias, conv_kernel, conv_bias, prelu_slope):
    from concourse.bass_utils import run_bass_kernel_spmd

    _install_compile_memo()
    slope = float(np.asarray(prelu_slope))
    need_lnsb = not (np.allclose(np.asarray(ln_scale), 1.0)
                     and np.allclose(np.asarray(ln_bias), 0.0))
    need_cb = not np.allclose(np.asarray(conv_bias), 0.0)

    cons_data = None
    cons_hash = None
    sq = 1.0
    if USE_INT8:
        cons_data = _make_consts(weights, conv_kernel)
        cons_hash = cons_data.tobytes()
        sq = _input_sigma(np.asarray(x, dtype=np.float32))
    key = (slope, need_lnsb, need_cb, USE_INT8, cons_hash, sq)
    if key not in _CACHE:
        _CACHE[key] = _build(slope, need_lnsb, need_cb, USE_INT8, cons_data, sq)
    nc = _CACHE[key]

    in_maps = _prep_inputs(x, weights, ln_scale, ln_bias, conv_kernel, conv_bias)
    res = run_bass_kernel_spmd(nc, in_maps, core_ids=list(range(8)))
    out = np.empty((B, T, C), np.float32)
    for i in range(B):
        if USE_INT8:
            o = np.asarray(res.results[i]["out"])  # (128, NOB) int8
            q = o[:, :OQ_SCL].reshape(128, NCH, 2, 4, 64).astype(np.float32)
            scl = (o[:, OQ_SCL:NOB].copy().view(np.float16)
                   .reshape(128, NCH, 2, 4).astype(np.float32))
            v = q * scl[..., None]           # (p, i, h, j, c)
            v = v.transpose(2, 1, 3, 0, 4)   # (h, i, j, p, c); t=h*4096+i*512+j*128+p
            out[i] = v.reshape(2 * HALF, 64)[:T]
        else:
            o = np.asarray(res.results[i]["out"])  # (128, HALF) f16
            full = o.astype(np.float32).reshape(2, 64, HALF).transpose(0, 2, 1)
            out[i] = full.reshape(2 * HALF, 64)[:T]
    return out
